# revision 1
# baseline (speedup 1.0000x reference)
"""Trainium2 Bass kernel: 2-layer MLP forward  y = relu(x@W1 + b1) @ W2 + b2.

Shapes: x [262144, 64], W1 [64, 128], b1 [128], W2 [128, 32], b2 [32].
Strategy (pure data parallel over 8 NeuronCores, 32768 rows each):

  * Host pre-transposes each x shard to feature-major xt [64, 32768] so the
    contraction dim lands on SBUF partitions (PE matmul contracts over the
    partition dim; a row-major x would otherwise need an on-chip transpose).
  * Device processes 2048-row super-chunks:
      - one 1 MiB DMA loads xt chunk as [128, 1024]: partitions 0-63 hold
        features of rows [C, C+1024), partitions 64-127 rows [C+1024, C+2048)
      - 4x matmul (K=64, alternating PE row-groups 0-1/2-3): lhsT = W1
        (stacked twice on 128 partitions), rhs = xt slices -> h_T in PSUM
      - ScalarE activation: relu(h + b1), PSUM -> SBUF
      - 4x matmul (K=128, col-tiled): lhsT = W2 at col-groups 0..3, each
        writing a 32-partition slice of one PSUM bank -> y_T stacked [128,512]
      - VectorE: + b2 (per-partition scalar), PSUM -> SBUF
      - DMA out to y_dev [16, 128, 512]
  * Matmuls run as float32r (1 col/cycle vs 4 for plain fp32; data is
    bit-identical fp32, only the instruction dtype differs via AP bitcast).
  * Host un-permutes y_dev back to [32768, 32] per shard and concatenates.
"""

import os
import sys

import numpy as np

if "/opt/trn_rl_repo" not in sys.path:
    sys.path.insert(0, "/opt/trn_rl_repo")

N_CORES = 8
B = 262144
B_C = B // N_CORES  # 32768
N_IN, N_MID, N_OUT = 64, 128, 32
CHUNK = 2048  # rows per super-chunk
QROWS = 512  # rows per matmul (PSUM bank free dim)
N_SC = B_C // CHUNK  # 16

# matmul instruction dtype: "f32r" (fast, ~tf32ish?) or "f32" (4x slower)
MM_DT = os.environ.get("BASS_MLP_MMDT", "f32r")

_CACHE: dict = {}


def _build_nc(mm_dt: str):
    from contextlib import ExitStack

    import concourse.bass as bass  # noqa: F401
    import concourse.tile as tile
    from concourse import bacc, mybir

    f32 = mybir.dt.float32
    bf16 = mybir.dt.bfloat16
    mmdt = {"f32r": mybir.dt.float32r, "f32": f32}[mm_dt]

    nc = bacc.Bacc(
        "TRN2", target_bir_lowering=False, debug=False, num_devices=N_CORES
    )
    # the x/W path is declared in the matmul dtype end-to-end (same 4-byte
    # fp32 payload for f32r; walrus requires matmul operands to be *produced*
    # as float32r, so the DMAs/activations must carry the tag).
    xt_d = nc.dram_tensor("xt", [N_IN, B_C], mmdt, kind="ExternalInput").ap()
    w1_d = nc.dram_tensor("w1", [N_IN, N_MID], mmdt, kind="ExternalInput").ap()
    b1_d = nc.dram_tensor("b1", [N_MID, 1], f32, kind="ExternalInput").ap()
    w2_d = nc.dram_tensor("w2", [N_MID, N_OUT], bf16, kind="ExternalInput").ap()
    b2s_d = nc.dram_tensor("b2s", [N_MID, 1], f32, kind="ExternalInput").ap()
    y_d = nc.dram_tensor(
        "y", [N_SC, N_MID, QROWS], f32, kind="ExternalOutput"
    ).ap()

    with tile.TileContext(nc) as tc, ExitStack() as ctx:
        consts = ctx.enter_context(tc.tile_pool(name="consts", bufs=1))
        x_pool = ctx.enter_context(tc.tile_pool(name="xp", bufs=4))
        h_pool = ctx.enter_context(tc.tile_pool(name="hp", bufs=6))
        y_pool = ctx.enter_context(tc.tile_pool(name="yp", bufs=4))
        hps_pool = ctx.enter_context(tc.tile_pool(name="hps", bufs=5, space="PSUM"))
        yps_pool = ctx.enter_context(tc.tile_pool(name="yps", bufs=2, space="PSUM"))

        # W1 stacked twice on the partition dim so row-groups 0-1 and 2-3 can
        # both serve K=64 matmuls whose rhs lives at base partition 0 / 64.
        w1_t = consts.tile([2 * N_IN, N_MID], mmdt, name="w1_t")
        nc.sync.dma_start(out=w1_t[0:N_IN, :], in_=w1_d)
        nc.sync.dma_start(out=w1_t[N_IN : 2 * N_IN, :], in_=w1_d)
        w2_t = consts.tile([N_MID, N_OUT], bf16, name="w2_t")
        nc.sync.dma_start(out=w2_t[:], in_=w2_d)
        b1_t = consts.tile([N_MID, 1], f32, name="b1_t")
        nc.sync.dma_start(out=b1_t[:], in_=b1_d)
        b2_t = consts.tile([N_MID, 1], f32, name="b2_t")
        nc.sync.dma_start(out=b2_t[:], in_=b2s_d)

        for s in range(N_SC):
            xt_t = x_pool.tile([128, CHUNK // 2], mmdt, name="xt_t", tag="xt")
            half_cols = CHUNK // 2
            for c in range(2):
                nc.sync.dma_start(
                    out=xt_t[64 * c : 64 * (c + 1), :],
                    in_=xt_d[:, s * CHUNK + c * half_cols : s * CHUNK + (c + 1) * half_cols],
                )

            y_ps = yps_pool.tile([128, QROWS], f32, name="y_ps", tag="y_ps")
            for q in range(4):
                c, half = q // 2, q % 2
                h_ps = hps_pool.tile([128, QROWS], f32, name="h_ps", tag="h_ps")
                rhs = xt_t[c * 64 : (c + 1) * 64, half * QROWS : (half + 1) * QROWS]
                lhsT = w1_t[c * 64 : (c + 1) * 64, :]
                nc.tensor.matmul(
                    h_ps[:], lhsT, rhs,
                    start=True, stop=True,
                )
                h_sb = h_pool.tile([128, QROWS], bf16, name="h_sb", tag="h_sb")
                nc.scalar.activation(
                    h_sb[:], h_ps[:],
                    mybir.ActivationFunctionType.Relu, bias=b1_t[:],
                )
                nc.tensor.matmul(
                    y_ps[32 * q : 32 * (q + 1), :],
                    w2_t[:], h_sb[:],
                    start=True, stop=True, tile_position=(0, 32 * q),
                )
            y_sb = y_pool.tile([128, QROWS], f32, name="y_sb", tag="y_sb")
            nc.vector.tensor_scalar_add(y_sb[:], y_ps[:], b2_t[:])
            nc.sync.dma_start(out=y_d[s], in_=y_sb[:])

    nc.compile()
    return nc


def _get_nc(mm_dt: str = MM_DT):
    if mm_dt not in _CACHE:
        _CACHE[mm_dt] = _build_nc(mm_dt)
    return _CACHE[mm_dt]


def _prep_in_maps(x, W1, b1, W2, b2):
    x = np.ascontiguousarray(x, dtype=np.float32)
    # [8, 64, B_C] feature-major shards
    xt = np.ascontiguousarray(x.reshape(N_CORES, B_C, N_IN).transpose(0, 2, 1))
    w1 = np.ascontiguousarray(W1, dtype=np.float32)
    import ml_dtypes
    w2 = np.ascontiguousarray(W2, dtype=np.float32).astype(ml_dtypes.bfloat16)
    b1c = np.ascontiguousarray(b1, dtype=np.float32).reshape(N_MID, 1)
    b2s = np.tile(np.asarray(b2, dtype=np.float32), 4).reshape(N_MID, 1)
    return [
        {"xt": xt[i], "w1": w1, "b1": b1c, "w2": w2, "b2s": b2s}
        for i in range(N_CORES)
    ]


def _unshard(results):
    outs = []
    for i in range(N_CORES):
        yd = results[i]["y"]  # [N_SC, 128, QROWS]
        # yd[s, 32q+o, j] = y[CHUNK*s + QROWS*q + j, o]
        y = (
            yd.reshape(N_SC, 4, N_OUT, QROWS)
            .transpose(0, 1, 3, 2)
            .reshape(B_C, N_OUT)
        )
        outs.append(y)
    return np.ascontiguousarray(np.concatenate(outs, axis=0))


def run(x, W1, b1, W2, b2, trace=False, mm_dt: str = MM_DT):
    from concourse.bass_utils import run_bass_kernel_spmd

    nc = _get_nc(mm_dt)
    in_maps = _prep_in_maps(x, W1, b1, W2, b2)
    res = run_bass_kernel_spmd(nc, in_maps, list(range(N_CORES)), trace=trace)
    return _unshard(res.results), res


def kernel(x, W1, b1, W2, b2):
    y, _ = run(x, W1, b1, W2, b2, trace=False)
    return y



# revision 2
# speedup vs baseline: 155.8403x; 155.8403x over previous
"""Trainium2 Bass kernel: 2-layer MLP forward  y = relu(x@W1 + b1) @ W2 + b2.

Shapes: x [262144, 64], W1 [64, 128], b1 [128], W2 [128, 32], b2 [32].
Strategy (pure data parallel over 8 NeuronCores, 32768 rows each):

  * All heavy tensors run in bf16 (x, W1, h, W2, y-out); accumulation is fp32
    in PSUM. Measured end-to-end rel err ~5e-3 (gate 2e-2). Host does the
    fp32->bf16 casts and the final bf16->fp32 upcast (not on the HW clock).
  * Host packs each 32768-row shard feature-major into xt2 [128, 16384]:
    partitions 0-63 hold the 64 features of rows [0, 16384) ("half A"),
    partitions 64-127 rows [16384, 32768) ("half B"). Every DMA then spans
    all 128 partitions -> all 16 SDMA engines pull their weight (a 64-
    partition DMA only engages 8 of them).
  * Per 1024-col chunk (2048 rows):
      - one 256 KiB DMA loads xt2[:, c*1024:(c+1)*1024] -> [128, 1024]
      - L1: 4 matmuls (K=64, N=512). Half-A uses PE row-group 0-1 (lhsT =
        W1 on partitions 0-63), half-B row-group 2-3; disjoint row-groups
        run concurrently on the PE -> ~2 cols/cycle aggregate.
      - ReLU+b1 split across engines: ScalarE activation handles the A-half
        [128,1024] PSUM tile, VectorE tensor_scalar (add b1, max 0) the
        B-half. Both write bf16 h to SBUF. fp32 PSUM reads are the per-lane
        throughput wall (1 elem/cycle/lane) so the two engines run in
        parallel on different PSUM banks.
      - L2: 4 matmuls (K=128, M=32, N=512), W2 col-tiled at tile_position
        (0, 32q) -> 4-way concurrent, y stacked [128, 512] in one PSUM bank.
      - +b2 (PSUM->SBUF, bf16): alternates ScalarE/VectorE per chunk to
        balance load.
      - one 128 KiB DMA stores y chunk.
  * Host un-permutes y [16, 128, 512] -> [32768, 32] per shard, upcasts.
"""

import sys

import numpy as np

if "/opt/trn_rl_repo" not in sys.path:
    sys.path.insert(0, "/opt/trn_rl_repo")

N_CORES = 8
B = 262144
B_C = B // N_CORES  # 32768 rows per core
HALF = B_C // 2  # 16384 rows per half
N_IN, N_MID, N_OUT = 64, 128, 32
CCOLS = 1024  # xt2 cols per chunk (= rows per half per chunk)
QROWS = 512  # rows per matmul (PSUM bank free dim)
N_CH = HALF // CCOLS  # 16 chunks

_CACHE: dict = {}


def _build_nc():
    from contextlib import ExitStack

    import concourse.bass as bass  # noqa: F401
    import concourse.tile as tile
    from concourse import bacc, mybir

    f32 = mybir.dt.float32
    bf16 = mybir.dt.bfloat16
    Alu = mybir.AluOpType

    nc = bacc.Bacc(
        "TRN2", target_bir_lowering=False, debug=False, num_devices=N_CORES
    )
    xt_d = nc.dram_tensor("xt", [2 * N_IN, HALF], bf16, kind="ExternalInput").ap()
    w1s_d = nc.dram_tensor("w1s", [2 * N_IN, N_MID], bf16, kind="ExternalInput").ap()
    b1_d = nc.dram_tensor("b1", [N_MID, 1], f32, kind="ExternalInput").ap()
    w2_d = nc.dram_tensor("w2", [N_MID, N_OUT], bf16, kind="ExternalInput").ap()
    b2s_d = nc.dram_tensor("b2s", [N_MID, 1], f32, kind="ExternalInput").ap()
    y_d = nc.dram_tensor(
        "y", [N_CH, N_MID, QROWS], bf16, kind="ExternalOutput"
    ).ap()

    with tile.TileContext(nc) as tc, ExitStack() as ctx:
        consts = ctx.enter_context(tc.tile_pool(name="consts", bufs=1))
        x_pool = ctx.enter_context(tc.tile_pool(name="xp", bufs=4))
        h_pool = ctx.enter_context(tc.tile_pool(name="hp", bufs=4))
        y_pool = ctx.enter_context(tc.tile_pool(name="yp", bufs=4))
        hps_pool = ctx.enter_context(tc.tile_pool(name="hps", bufs=3, space="PSUM"))
        yps_pool = ctx.enter_context(tc.tile_pool(name="yps", bufs=2, space="PSUM"))

        # W1 twice on the partition dim: rows 0-63 serve PE row-group 0-1
        # (half A), rows 64-127 row-group 2-3 (half B).
        w1_t = consts.tile([2 * N_IN, N_MID], bf16, name="w1_t")
        nc.sync.dma_start(out=w1_t[:], in_=w1s_d)
        w2_t = consts.tile([N_MID, N_OUT], bf16, name="w2_t")
        nc.sync.dma_start(out=w2_t[:], in_=w2_d)
        b1_t = consts.tile([N_MID, 1], f32, name="b1_t")
        nc.sync.dma_start(out=b1_t[:], in_=b1_d)
        b2_t = consts.tile([N_MID, 1], f32, name="b2_t")
        nc.sync.dma_start(out=b2_t[:], in_=b2s_d)

        for c in range(N_CH):
            xt_t = x_pool.tile([128, CCOLS], bf16, name="xt_t", tag="xt")
            nc.sync.dma_start(
                out=xt_t[:], in_=xt_d[:, c * CCOLS : (c + 1) * CCOLS]
            )

            hA_ps = hps_pool.tile([128, CCOLS], f32, name="hA_ps", tag="hps")
            hB_ps = hps_pool.tile([128, CCOLS], f32, name="hB_ps", tag="hps")
            # interleave A/B so consecutive matmuls hit disjoint row-groups
            for half in range(2):
                nc.tensor.matmul(
                    hA_ps[:, half * QROWS : (half + 1) * QROWS],
                    w1_t[0:N_IN, :],
                    xt_t[0:N_IN, half * QROWS : (half + 1) * QROWS],
                    start=True, stop=True,
                )
                nc.tensor.matmul(
                    hB_ps[:, half * QROWS : (half + 1) * QROWS],
                    w1_t[N_IN : 2 * N_IN, :],
                    xt_t[N_IN : 2 * N_IN, half * QROWS : (half + 1) * QROWS],
                    start=True, stop=True,
                )

            hA_sb = h_pool.tile([128, CCOLS], bf16, name="hA_sb", tag="hsb")
            hB_sb = h_pool.tile([128, CCOLS], bf16, name="hB_sb", tag="hsb")
            nc.scalar.activation(
                hA_sb[:], hA_ps[:],
                mybir.ActivationFunctionType.Relu, bias=b1_t[:],
            )
            nc.vector.tensor_scalar(
                hB_sb[:], hB_ps[:], b1_t[:], 0.0, Alu.add, Alu.max
            )

            y_ps = yps_pool.tile([128, QROWS], f32, name="y_ps", tag="yps")
            for q in range(4):
                h_sb = hA_sb if q < 2 else hB_sb
                half = q % 2
                nc.tensor.matmul(
                    y_ps[32 * q : 32 * (q + 1), :],
                    w2_t[:],
                    h_sb[:, half * QROWS : (half + 1) * QROWS],
                    start=True, stop=True, tile_position=(0, 32 * q),
                )

            y_sb = y_pool.tile([128, QROWS], bf16, name="y_sb", tag="ysb")
            if c % 2 == 0:
                nc.scalar.activation(
                    y_sb[:], y_ps[:],
                    mybir.ActivationFunctionType.Identity, bias=b2_t[:],
                )
            else:
                nc.vector.tensor_scalar_add(y_sb[:], y_ps[:], b2_t[:])
            nc.sync.dma_start(out=y_d[c], in_=y_sb[:])

    nc.compile()
    return nc


def _get_nc():
    if "nc" not in _CACHE:
        _CACHE["nc"] = _build_nc()
    return _CACHE["nc"]


def _prep_in_maps(x, W1, b1, W2, b2):
    import ml_dtypes

    bf = ml_dtypes.bfloat16
    x = np.ascontiguousarray(x, dtype=np.float32)
    # [8, 2, 64, 16384]: per core, half-major feature-major
    xt = np.ascontiguousarray(
        x.astype(bf).reshape(N_CORES, 2, HALF, N_IN).transpose(0, 1, 3, 2)
    ).reshape(N_CORES, 2 * N_IN, HALF)
    w1 = np.asarray(W1, dtype=np.float32).astype(bf)
    w1s = np.ascontiguousarray(np.concatenate([w1, w1], axis=0))
    w2 = np.ascontiguousarray(np.asarray(W2, dtype=np.float32).astype(bf))
    b1c = np.ascontiguousarray(b1, dtype=np.float32).reshape(N_MID, 1)
    b2s = np.tile(np.asarray(b2, dtype=np.float32), 4).reshape(N_MID, 1)
    return [
        {"xt": xt[i], "w1s": w1s, "b1": b1c, "w2": w2, "b2s": b2s}
        for i in range(N_CORES)
    ]


def _unshard(results):
    outs = []
    for i in range(N_CORES):
        yd = results[i]["y"]  # [N_CH, 128, 512] bf16
        # yd[c, 32q+o, j] -> row = (q//2)*HALF + c*CCOLS + (q%2)*QROWS + j
        y = (
            yd.reshape(N_CH, 2, 2, N_OUT, QROWS)  # c, halfsel, sub, o, j
            .transpose(1, 0, 2, 4, 3)  # halfsel, c, sub, j, o
            .reshape(B_C, N_OUT)
        )
        outs.append(y)
    return np.ascontiguousarray(
        np.concatenate(outs, axis=0).astype(np.float32)
    )


def run(x, W1, b1, W2, b2, trace=False):
    from concourse.bass_utils import run_bass_kernel_spmd

    nc = _get_nc()
    in_maps = _prep_in_maps(x, W1, b1, W2, b2)
    res = run_bass_kernel_spmd(nc, in_maps, list(range(N_CORES)), trace=trace)
    return _unshard(res.results), res


def kernel(x, W1, b1, W2, b2):
    y, _ = run(x, W1, b1, W2, b2, trace=False)
    return y
